# revision 17
# baseline (speedup 1.0000x reference)
"""Causal attention decoder block on 8 trn2 NeuronCores.

Sharding: core = (batch b in 0..1, head-group g in 0..3); each core computes
4 heads of one batch element: QKV projection slices, RoPE, causal attention,
and a partial output projection (its heads' rows of Wout). Host sums the 4
partials per batch and adds bout.

v2 schedule notes (vs the first working version):
  - Inputs stream over 4 DMA queues; xt is one [128, 8*2048] SBUF tile whose
    column-quarters arrive k-major per queue, so the ch-major QK projection
    follows the DMA arrival order with no long initial stall.
  - K^T is projected first, then Q^T's chunk-1 columns, so chunk-1 attention
    (scores+exp) is emitted mid-phase-1 and the scalar engine starts its exp
    stream ~20us earlier.
  - Attention is software-pipelined: pv matmuls trail their scores by LOOK
    tiles so the PE never waits on the scalar exp round-trip (keeps the PE
    p-state at full clock).
  - Scalar engine does exp only (plus idle-phase rope casts); all relayout /
    staging copies live on gpsimd (Pool); reciprocal uses the fast DVE
    approximation straight from PSUM; rope runs in bf16 for the 2x DVE mode.
"""
import ml_dtypes
import numpy as np

import concourse.bass as bass
import concourse.mybir as mybir
from concourse import bacc
from concourse.ap import AP
from concourse.tile import TileContext

F32 = mybir.dt.float32
F32R = mybir.dt.float32r
BF16 = mybir.dt.bfloat16
EXP = mybir.ActivationFunctionType.Exp

B, N, D = 2, 2048, 1024
H, HD = 16, 64
HPG = 4               # heads per group/core
C = HPG * HD          # 256 cols per core per tensor
SCALE = HD ** -0.5
ROPE_BASE = 10000.0
NT = N // 128         # 16 seq tiles
NCH = N // 512        # 4 seq chunks
KT = D // 128         # 8 contraction tiles
MBIG = -1e9
LOOK = 2              # pv lookahead depth (tiles)

# ---------------------------------------------------------------- host tables

def _host_tables():
    perm = np.zeros(HD, np.int64)
    freqi = np.zeros(HD, np.int64)
    sign = np.zeros(HD, np.float32)
    for c in range(HD):
        q, r = divmod(c, 32)
        s, j = divmod(r, 16)
        i = q * 16 + j
        perm[c] = 2 * i + s
        freqi[c] = i
        sign[c] = -1.0 if s == 0 else 1.0
    inv_freq = 1.0 / (ROPE_BASE ** (np.arange(0, HD, 2, dtype=np.float32) / HD))
    ang = np.outer(inv_freq[freqi], np.arange(N, dtype=np.float32))   # (64, N)
    cos2 = np.tile(np.cos(ang).astype(np.float32), (2, 1))            # (128, N)
    sin2 = np.tile((np.sin(ang) * sign[:, None]).astype(np.float32), (2, 1))
    # triangular tile: element (m, q) masks scores with q < m
    m = np.arange(128)[:, None]
    q = np.arange(128)[None, :]
    tri = np.where(q >= m, 0.0, MBIG).astype(np.float32)
    ident = np.eye(128, dtype=np.float32)
    return perm, cos2, sin2, tri, ident

_PERM, _COS2, _SIN2, _TRI, _IDENT = _host_tables()
_SHUF_MASK = [(i ^ 16) for i in range(32)]
# selector for broadcasting the per-chunk 1/rowsum rows (4 rows, row = head)
# to a 128-partition head-pair tile: block t rows 0-63 <- head 2t, 64-127 <-
# head 2t+1
_SEL = np.zeros((1, 256), np.float32)
_SEL[0, 0:64] = 1.0
_SEL[0, 128 + 64:128 + 128] = 1.0

# ---------------------------------------------------------------- bass kernel

def build_nc(dbg=False):
    nc = bacc.Bacc("TRN2", target_bir_lowering=False, debug=False)
    xt_d = nc.dram_tensor("xt", [D, N], BF16, kind="ExternalInput").ap()
    wq_d = nc.dram_tensor("wq", [D, C], BF16, kind="ExternalInput").ap()
    wk_d = nc.dram_tensor("wk", [D, C], BF16, kind="ExternalInput").ap()
    wv_d = nc.dram_tensor("wv", [D, C], BF16, kind="ExternalInput").ap()
    wout_d = nc.dram_tensor("wout", [C, D], BF16, kind="ExternalInput").ap()
    cos_d = nc.dram_tensor("cos2", [128, N], BF16, kind="ExternalInput").ap()
    sin_d = nc.dram_tensor("sin2", [128, N], BF16, kind="ExternalInput").ap()
    tri_d = nc.dram_tensor("tri", [128, 128], BF16, kind="ExternalInput").ap()
    id_d = nc.dram_tensor("ident", [128, 128], BF16, kind="ExternalInput").ap()
    sel_d = nc.dram_tensor("sel", [1, 256], BF16, kind="ExternalInput").ap()
    out_d = nc.dram_tensor("out", [N, D], F32, kind="ExternalOutput").ap()
    if dbg:
        dxt_d = nc.dram_tensor("dxt", [128, KT * N], BF16,
                               kind="ExternalOutput").ap()
        dqr_d = nc.dram_tensor("dqr", [128, N], BF16,
                               kind="ExternalOutput").ap()
        dkr_d = nc.dram_tensor("dkr", [128, N], BF16,
                               kind="ExternalOutput").ap()
        do_d = nc.dram_tensor("do", [128, N], BF16,
                              kind="ExternalOutput").ap()
        dva_d = nc.dram_tensor("dva", [128, HPG * (HD + 1)], BF16,
                               kind="ExternalOutput").ap()

    with TileContext(nc) as tc:
        with tc.tile_pool(name="persist", bufs=1) as pp, \
             tc.tile_pool(name="scr", bufs=4) as sp, \
             tc.tile_pool(name="psbig", bufs=3, space="PSUM") as bigp, \
             tc.tile_pool(name="pspv", bufs=2, space="PSUM") as pvp, \
             tc.tile_pool(name="psbc", bufs=1, space="PSUM") as bcp:

            # ---- persistent tiles
            xt_sb = pp.tile([128, KT * N], BF16, tag="xt", name="xt")
            qr_sb = [pp.tile([128, N], BF16, tag=f"qr{t}", name=f"qr{t}")
                     for t in range(2)]
            kr_sb = [pp.tile([128, N], BF16, tag=f"kr{t}", name=f"kr{t}")
                     for t in range(2)]
            vaug_sb = [pp.tile([128, HPG * (HD + 1)], BF16, tag=f"va{i}",
                               name=f"va{i}") for i in range(NT)]
            o_sb = [pp.tile([128, N], BF16, tag=f"o{t}", name=f"o{t}")
                    for t in range(2)]
            rrh_sb = {}  # (qc, hl) -> [1,512] bf16 rinv row
            wq_sb = pp.tile([128, KT * C], BF16, tag="wq", name="wq")
            wk_sb = pp.tile([128, KT * C], BF16, tag="wk", name="wk")
            wv_sb = pp.tile([128, KT * C], BF16, tag="wv", name="wv")
            wout_sb = [pp.tile([128, D], BF16, tag=f"wout{t}", name=f"wo{t}")
                       for t in range(2)]
            cos_sb = pp.tile([128, N], BF16, tag="cos", name="cos")
            sin_sb = pp.tile([128, N], BF16, tag="sin", name="sin")
            tri_sb = pp.tile([128, 128], BF16, tag="tri", name="tri")
            id_sb = pp.tile([128, 128], BF16, tag="ident", name="ident")
            sel_sb = pp.tile([1, 256], BF16, tag="sel", name="sel")
            warm = pp.tile([1, 16], F32, tag="warm", name="warm")

            # ---- input DMA program (4 queues; k-major per column-quarter)
            def dma_w(queue, w_sbuf, w_dram, k0, k1):
                dst = AP(w_sbuf.tensor, w_sbuf.offset + k0 * C,
                         [[KT * C, 128], [C, k1 - k0], [1, C]])
                src = AP(w_dram.tensor, k0 * 128 * C,
                         [[C, 128], [128 * C, k1 - k0], [1, C]])
                queue.dma_start(dst, src)

            def dma_xt(queue, ch, k0, k1):
                dst = AP(xt_sb.tensor, xt_sb.offset + k0 * N + ch * 512,
                         [[KT * N, 128], [N, k1 - k0], [1, 512]])
                src = AP(xt_d.tensor, k0 * 128 * N + ch * 512,
                         [[N, 128], [128 * N, k1 - k0], [1, 512]])
                queue.dma_start(dst, src)

            # queue 0 (sync): wk then xt ch0 (K proj goes first), tri/id
            # early (chunk-0 attention is emitted first), then wv
            dma_w(nc.sync, wk_sb, wk_d, 0, 2)
            dma_w(nc.sync, wk_sb, wk_d, 2, 8)
            dma_xt(nc.sync, 0, 0, 2)
            dma_xt(nc.sync, 0, 2, 4)
            dma_xt(nc.sync, 0, 4, 8)
            # queue 1 (scalar): wq, mask tables, xt ch1, wout
            dma_w(nc.scalar, wq_sb, wq_d, 0, 2)
            dma_w(nc.scalar, wq_sb, wq_d, 2, 8)
            nc.scalar.dma_start(tri_sb[:], tri_d[:])
            nc.scalar.dma_start(id_sb[:], id_d[:])
            nc.scalar.dma_start(sel_sb[:], sel_d[:])
            dma_xt(nc.scalar, 1, 0, 4)
            dma_xt(nc.scalar, 1, 4, 8)
            for t in range(2):
                nc.scalar.dma_start(wout_sb[t][:],
                                    wout_d[t * 128:(t + 1) * 128, :])
            # queue 2 (gpsimd): cos/sin interleaved with xt ch2/ch3, wv
            nc.gpsimd.dma_start(cos_sb[:], cos_d[:])
            dma_xt(nc.gpsimd, 2, 0, 4)
            nc.gpsimd.dma_start(sin_sb[:], sin_d[:])
            dma_xt(nc.gpsimd, 3, 0, 4)
            dma_xt(nc.gpsimd, 2, 4, 8)
            dma_xt(nc.gpsimd, 3, 4, 8)
            dma_w(nc.gpsimd, wv_sb, wv_d, 0, 8)

            # ---- warmup (exp act table) + vaug ones columns (after DMA
            # issues so the queue programs start moving data immediately)
            nc.gpsimd.memset(warm[:], 1.0)
            we = sp.tile([1, 16], BF16, tag="we", bufs=1, name="we")
            nc.scalar.activation(we[:], warm[:], EXP, scale=1.0)
            for i in range(NT):
                ap = vaug_sb[i][:]
                ones_ap = AP(ap.tensor, ap.offset + HD,
                             [[HPG * (HD + 1), 128], [HD + 1, HPG]])
                nc.gpsimd.memset(ones_ap, 1.0)

            # ---- phase 1a: QK projection (ch-major) + rope
            def qk_group(w_sbuf, dst, mt, ch, cast_pool):
                ps = bigp.tile([128, 512], F32, tag="big", name="qkps")
                for k in range(KT):
                    nc.tensor.matmul(
                        ps[:],
                        w_sbuf[:, k * C + mt * 128:k * C + (mt + 1) * 128],
                        xt_sb[:, k * N + ch * 512:k * N + ch * 512 + 512],
                        start=(k == 0), stop=(k == KT - 1))
                xb = sp.tile([128, 512], BF16, tag="xb", bufs=3, name="xb")
                if cast_pool:
                    nc.vector.tensor_copy(xb[:], ps[:])
                else:
                    nc.scalar.copy(xb[:], ps[:])
                xs = sp.tile([128, 512], BF16, tag="xs", bufs=3, name="xs")
                nc.vector.stream_shuffle(xs[:], xb[:], _SHUF_MASK)
                cs = cos_sb[:, ch * 512:(ch + 1) * 512]
                sn = sin_sb[:, ch * 512:(ch + 1) * 512]
                m2 = sp.tile([128, 512], BF16, tag="mm", bufs=4, name="m2")
                nc.vector.tensor_mul(m2[:], xs[:], sn)
                m1 = sp.tile([128, 512], BF16, tag="mm", bufs=4, name="m1")
                nc.vector.tensor_mul(m1[:], xb[:], cs)
                nc.vector.tensor_add(
                    dst[mt][:, ch * 512:(ch + 1) * 512], m1[:], m2[:])

            for mt in range(2):
                qk_group(wk_sb, kr_sb, mt, 0, cast_pool=False)
            for mt in range(2):
                qk_group(wq_sb, qr_sb, mt, 0, cast_pool=False)

            # ---- attention machinery (software-pipelined pv)
            pending = []
            head_state = {}

            def drain(n):
                for _ in range(min(n, len(pending))):
                    pending.pop(0)()

            def emit_tile(qc, hl, mt, hold=False):
                nmt = 4 * (qc + 1)
                t = hl // 2
                pb = (hl % 2) * 64
                v = mt - 4 * qc
                q0 = 128 * v if v > 0 else 0
                s_ps = bigp.tile([128, 512], F32, tag="big", name="sps")
                nc.tensor.matmul(
                    s_ps[:, q0:512],
                    kr_sb[t][pb:pb + 64, mt * 128:(mt + 1) * 128],
                    qr_sb[t][pb:pb + 64, qc * 512 + q0:(qc + 1) * 512],
                    start=True, stop=(v < 0))
                if v >= 0:
                    nc.tensor.matmul(
                        s_ps[:, q0:q0 + 128], id_sb[:], tri_sb[:],
                        start=False, stop=True)
                e_sb = sp.tile([128, 512], BF16, tag="e", name="e", bufs=8)
                nc.scalar.activation(e_sb[:, q0:512], s_ps[:, q0:512],
                                     EXP, scale=SCALE)

                def pv_op(qc=qc, hl=hl, mt=mt, q0=q0, t=t, pb=pb,
                          first=(mt == 0), last=(mt == nmt - 1), e_sb=e_sb):
                    if first:
                        head_state[(qc, hl)] = pvp.tile(
                            [HD + 1, 512], F32, tag="pv", name="pv")
                    pv = head_state[(qc, hl)]
                    nc.tensor.matmul(
                        pv[:, q0:512],
                        vaug_sb[mt][:, hl * (HD + 1):(hl + 1) * (HD + 1)],
                        e_sb[:, q0:512],
                        start=first, stop=last)
                    if last:
                        sr = sp.tile([1, 512], F32, tag="sr", bufs=2,
                                     name="sr")
                        nc.vector.tensor_copy(sr[:], pv[64:65, :])
                        rv = sp.tile([1, 512], F32, tag="rv", bufs=2,
                                     name="rv")
                        nc.vector.reciprocal_approx_fast(rv[:], sr[:])
                        rrh = sp.tile([1, 512], BF16, tag="rrh", bufs=4,
                                      name="rrh")
                        nc.vector.tensor_copy(rrh[:], rv[:])
                        rrh_sb[(qc, hl)] = rrh

                pending.append(pv_op)
                if not hold and len(pending) > LOOK:
                    pending.pop(0)()

            def emit_head(qc, hl, fillers=None, hold=False):
                for mt in range(4 * (qc + 1)):
                    emit_tile(qc, hl, mt, hold=hold)
                    if fillers and mt % 5 == 4:
                        fillers.pop(0)()

            qs = [nc.gpsimd, nc.sync, nc.gpsimd]
            dma_rot = [0]

            def norm_piece(qc, t):
                # bc: broadcast 1/rowsum rows of head pair t to 128 partitions
                bc = bcp.tile([128, 512], F32, tag="bc", name="bc")
                for j in range(2):
                    nc.tensor.matmul(
                        bc[:], sel_sb[0:1, j * 128:(j + 1) * 128],
                        rrh_sb[(qc, 2 * t + j)][:],
                        start=(j == 0), stop=(j == 1))
                for half, hl in ((0, 2 * t), (1, 2 * t + 1)):
                    rbc = sp.tile([64, 512], BF16, tag="rbc", bufs=2,
                                  name="rbc")
                    nc.vector.tensor_copy(rbc[:], bc[half * 64:half * 64 + 64, :])
                    pv = head_state[(qc, hl)]
                    nc.vector.tensor_mul(
                        o_sb[t][half * 64:half * 64 + 64,
                                qc * 512:(qc + 1) * 512],
                        pv[0:64, :], rbc[:])

            def proj_piece(pqc, j, pjp):
                for i in (4 * pqc + 2 * j, 4 * pqc + 2 * j + 1):
                    for cc in range(2):
                        ps = pjp.tile([128, 512], F32, tag="pj", name="op")
                        for t in range(2):
                            nc.tensor.matmul(
                                ps[:],
                                o_sb[t][:, i * 128:(i + 1) * 128],
                                wout_sb[t][:, cc * 512:(cc + 1) * 512],
                                start=(t == 0), stop=(t == 1))
                        oc = sp.tile([128, 512], F32, tag="oc", bufs=3,
                                     name="oc")
                        nc.vector.tensor_copy(oc[:], ps[:])
                        qs[dma_rot[0] % 3].dma_start(
                            out_d[i * 128:(i + 1) * 128,
                                  cc * 512:(cc + 1) * 512], oc[:])
                        dma_rot[0] += 1

            # ---- chunk 0 attention emitted early (Act starts exp'ing),
            # pv held until V tiles exist; remaining Q groups use DVE casts
            # (Act queue is already full of exps)
            emit_head(0, 0, hold=True)
            for ch in (1, 2, 3):
                for mt in range(2):
                    qk_group(wk_sb, kr_sb, mt, ch, cast_pool=True)
                for mt in range(2):
                    qk_group(wq_sb, qr_sb, mt, ch, cast_pool=True)

            # ---- phase 1b: V projection + chunk-0 attention interleave
            with tc.tile_pool(name="psv", bufs=2, space="PSUM") as vp:
                def v_group(i):
                    ps = vp.tile([128, 512], F32, tag="v", name="vps")
                    for k in range(KT):
                        nc.tensor.matmul(
                            ps[:, 0:C],
                            xt_sb[:, k * N + i * 128:k * N + (i + 1) * 128],
                            wv_sb[:, k * C:(k + 1) * C],
                            start=(k == 0), stop=(k == KT - 1))
                    ap = vaug_sb[i][:]
                    dst = AP(ap.tensor, ap.offset,
                             [[HPG * (HD + 1), 128], [HD + 1, HPG], [1, HD]])
                    nc.scalar.copy(
                        dst, ps[:, 0:C].rearrange("p (a c) -> p a c",
                                                  a=HPG, c=HD))

                for i in range(4):
                    v_group(i)
                drain(len(pending) - LOOK)
                emit_head(0, 1)
                drain(len(pending))
                norm_piece(0, 0)
                v_group(4)
                v_group(5)
                emit_head(0, 2)
                v_group(6)
                v_group(7)
                v_group(8)
                v_group(9)
                emit_head(0, 3)
                drain(len(pending))
                norm_piece(0, 1)
                for i in range(10, 16):
                    v_group(i)

            # ---- chunks 1, 2, 3 with previous-chunk proj injected as
            # fillers inside the heads (keeps the PE fed while Act paces)
            with tc.tile_pool(name="pspj", bufs=2, space="PSUM") as pjp:
                def proj_fillers(pqc):
                    fl = []
                    for i in range(4 * pqc, 4 * pqc + 4):
                        for cc in range(2):
                            def f(i=i, cc=cc):
                                ps = pjp.tile([128, 512], F32, tag="pj",
                                              name="op")
                                for t in range(2):
                                    nc.tensor.matmul(
                                        ps[:],
                                        o_sb[t][:, i * 128:(i + 1) * 128],
                                        wout_sb[t][:, cc * 512:(cc + 1) * 512],
                                        start=(t == 0), stop=(t == 1))
                                oc = sp.tile([128, 512], F32, tag="oc",
                                             bufs=3, name="oc")
                                nc.vector.tensor_copy(oc[:], ps[:])
                                qs[dma_rot[0] % 3].dma_start(
                                    out_d[i * 128:(i + 1) * 128,
                                          cc * 512:(cc + 1) * 512], oc[:])
                                dma_rot[0] += 1
                            fl.append(f)
                    return fl

                for cur in (1, 2, 3):
                    fillers = proj_fillers(cur - 1)
                    emit_head(cur, 0, fillers)
                    emit_head(cur, 1, fillers)
                    drain(len(pending))
                    norm_piece(cur, 0)
                    emit_head(cur, 2, fillers)
                    emit_head(cur, 3, fillers)
                    drain(len(pending))
                    norm_piece(cur, 1)
                    for f in fillers:
                        f()
                for f in proj_fillers(3):
                    f()
                if dbg:
                    nc.sync.dma_start(dxt_d[:], xt_sb[:])
                    nc.sync.dma_start(dqr_d[:], qr_sb[0][:])
                    nc.sync.dma_start(dkr_d[:], kr_sb[0][:])
                    nc.sync.dma_start(do_d[:], o_sb[1][:])
                    nc.sync.dma_start(dva_d[:], vaug_sb[15][:])

    nc.compile()
    return nc


# ---------------------------------------------------------------- host wrapper

_NC = None


def make_in_maps(X, Wqkv, Wout, bout):
    X = np.ascontiguousarray(np.asarray(X, np.float32))
    Wqkv = np.asarray(Wqkv, np.float32)
    Wout = np.asarray(Wout, np.float32)
    in_maps = []
    for core in range(8):
        b, g = core // 4, core % 4
        heads = [HPG * g + hl for hl in range(HPG)]
        qcols = np.concatenate([h * HD + _PERM for h in heads])
        vcols = np.concatenate([h * HD + np.arange(HD) for h in heads])
        in_maps.append({
            "xt": np.ascontiguousarray(X[b].T).astype(ml_dtypes.bfloat16),
            "wq": np.ascontiguousarray(Wqkv[:, qcols]).astype(ml_dtypes.bfloat16),
            "wk": np.ascontiguousarray(Wqkv[:, 1024 + qcols]).astype(ml_dtypes.bfloat16),
            "wv": np.ascontiguousarray(Wqkv[:, 2048 + vcols]).astype(ml_dtypes.bfloat16),
            "wout": np.ascontiguousarray(Wout[vcols, :]).astype(ml_dtypes.bfloat16),
            "cos2": _COS2.astype(ml_dtypes.bfloat16),
            "sin2": _SIN2.astype(ml_dtypes.bfloat16),
            "tri": _TRI.astype(ml_dtypes.bfloat16),
            "ident": _IDENT.astype(ml_dtypes.bfloat16),
            "sel": _SEL.astype(ml_dtypes.bfloat16),
        })
    return in_maps


def assemble(results, bout):
    out = np.zeros((B, N, D), np.float32)
    for core in range(8):
        out[core // 4] += results[core]["out"]
    out += np.asarray(bout, np.float32)[None, None, :]
    return out


def kernel(X, Wqkv, Wout, bout):
    global _NC
    from concourse import bass_utils
    if _NC is None:
        _NC = build_nc()
    in_maps = make_in_maps(X, Wqkv, Wout, bout)
    res = bass_utils.run_bass_kernel_spmd(_NC, in_maps, core_ids=list(range(8)))
    return assemble(res.results, bout)
